# revision 3
# baseline (speedup 1.0000x reference)
"""Trainium2 Bass kernel: distance-decay double-softmax attention.

Reference computation per (b, c) pair (L=256, D=512):
    qkv  = x @ w_qkv;  q,k,v = split(qkv)
    attn = softmax(q @ k.T * D_h^-0.5)
    h    = relu((attn + pos) @ w1 + b1);  w = h @ w2 + b2
    attn2= softmax(attn * exp(-dist / (2 w^2 + 1e-6)))
    out  = (attn2 @ v) @ w_out + b_out

Sharding: pure data parallel over the 128 (b,c) pairs -> 16 pairs/core,
packed as 8 "superpairs" (2 batch items of one channel share the free
dim, giving N=512 matmuls).  Device layouts are chosen so no on-device
transposes are needed except attn_w^T / attn2^T (PE transpose via
identity); x arrives host-pretransposed and the output leaves as y^T,
untransposed on the host.

All matmuls run as float32r (full-rate fp32 storage); everything
accumulates in fp32 PSUM.
"""

import sys
import numpy as np

sys.path.insert(0, "/opt/trn_rl_repo")

import concourse.bass as bass  # noqa: E402
import concourse.mybir as mybir  # noqa: E402
from concourse import bacc  # noqa: E402
from concourse.tile import TileContext  # noqa: E402

F32 = mybir.dt.float32
F32R = mybir.dt.float32r
AF = mybir.ActivationFunctionType
ALU = mybir.AluOpType

B, C, L, D = 8, 16, 256, 512
NCORES = 8
CH_PER_CORE = C // NCORES          # 2
NSP = (B // 2) * CH_PER_CORE       # 8 superpairs per core
P = 128
FP = 2 * L                         # 512: two pairs packed along free dim
DT = D // P                        # 4
LT = L // P                        # 2
SCALE = float(64 ** -0.5)          # DIM_HEAD ** -0.5


def _r(ap):
    return ap


def _emit(nc, tc, h):
    MM = nc.tensor.matmul

    import contextlib
    with contextlib.ExitStack() as ex:
        cpool = ex.enter_context(tc.tile_pool(name="consts", bufs=1))
        sp_pool = ex.enter_context(tc.tile_pool(name="stream", bufs=2))
        ypool = ex.enter_context(tc.tile_pool(name="yout", bufs=1))
        pp = ex.enter_context(tc.tile_pool(name="ps", bufs=8, space="PSUM"))

        # ---- constants (loaded once) ----
        wq_sb = []
        for dt in range(DT):
            t = cpool.tile([P, 3 * D], F32R, tag=f"wqkv{dt}", name=f"wqkv{dt}")
            nc.sync.dma_start(out=t[:, :], in_=h["wqkv"][dt * P:(dt + 1) * P, :])
            wq_sb.append(t)
        w1_sb = []
        for mt in range(LT):
            t = cpool.tile([P, L], F32R, tag=f"w1_{mt}", name=f"w1_{mt}")
            nc.sync.dma_start(out=t[:, :], in_=h["w1"][mt * P:(mt + 1) * P, :])
            w1_sb.append(t)
        w2_sb = []
        for jt in range(LT):
            t = cpool.tile([P, 2], F32R, tag=f"w2_{jt}", name=f"w2_{jt}")
            nc.sync.dma_start(out=t[:, :], in_=h["w2d"][jt * P:(jt + 1) * P, :])
            w2_sb.append(t)
        b1_sb = cpool.tile([1, L], F32R, tag="b1r", name="b1r")
        nc.sync.dma_start(out=b1_sb[0:1, :], in_=h["b1r"][0:1, :])
        ones_sb = cpool.tile([1, FP], F32R, tag="ones", name="ones")
        nc.sync.dma_start(out=ones_sb[0:1, :], in_=h["ones"][0:1, :])
        wo_sb = []
        for dt in range(DT):
            t = cpool.tile([P, D], F32R, tag=f"wout{dt}", name=f"wout{dt}")
            nc.sync.dma_start(out=t[:, :], in_=h["wout"][dt * P:(dt + 1) * P, :])
            wo_sb.append(t)
        bo_sb = []
        for ot in range(DT):
            t = cpool.tile([P, 1], F32, tag=f"bout{ot}", name=f"bout{ot}")
            nc.sync.dma_start(out=t[:, :], in_=h["bout"][ot * P:(ot + 1) * P, :])
            bo_sb.append(t)
        b2_sb = cpool.tile([P, 1], F32, tag="b2r", name="b2r")
        nc.sync.dma_start(out=b2_sb[:, :], in_=h["b2r"][:, :])
        id_sb = cpool.tile([P, P], F32, tag="ident", name="ident")
        nc.sync.dma_start(out=id_sb[:, :], in_=h["ident"][:, :])
        dist_sb = []
        for it in range(LT):
            t = cpool.tile([P, FP], F32, tag=f"dist{it}", name=f"dist{it}")
            nc.sync.dma_start(out=t[:, :], in_=h["dist"][it * P:(it + 1) * P, :])
            dist_sb.append(t)
        pos_sb = []
        for ci in range(CH_PER_CORE):
            row = []
            for it in range(LT):
                t = cpool.tile([P, FP], F32, tag=f"pos{ci}_{it}", name=f"pos{ci}_{it}")
                nc.sync.dma_start(out=t[:, :], in_=h["pos"][ci, it * P:(it + 1) * P, :])
                row.append(t)
            pos_sb.append(row)

        # ---- per-superpair pipeline ----
        for sp in range(NSP):
            ci = sp // (NSP // CH_PER_CORE)

            # S1: load x^T (4 tiles [128(d), 512(l packed)])
            xt = []
            for dt in range(DT):
                t = sp_pool.tile([P, FP], F32R, tag=f"xt{dt}", name=f"xt{sp}_{dt}")
                nc.sync.dma_start(out=t[:, :], in_=h["x_t"][sp, dt * P:(dt + 1) * P, :])
                xt.append(t)

            # S2: qkv.  q^T,k^T: [e(part), l(packed free)]; v: [l(part), e(free)]
            qT, kT = [], []
            for which, dst in (("q", qT), ("k", kT)):
                off = 0 if which == "q" else D
                for et in range(DT):
                    ps = pp.tile([P, FP], F32, tag="ps", name=f"ps_{which}{sp}_{et}")
                    for dt in range(DT):
                        MM(ps[:, :],
                           _r(wq_sb[dt][:, off + et * P: off + (et + 1) * P]),
                           _r(xt[dt][:, :]),
                           start=(dt == 0), stop=(dt == DT - 1))
                    t = sp_pool.tile([P, FP], F32R, tag=f"{which}T{et}",
                                     name=f"{which}T{sp}_{et}")
                    nc.vector.tensor_copy(t[:, :], ps[:, :])
                    dst.append(t)
            v_sb = [[None] * LT for _ in range(2)]
            for pi in range(2):
                for lt in range(LT):
                    ps = pp.tile([P, D], F32, tag="ps", name=f"ps_v{sp}_{pi}{lt}")
                    for dt in range(DT):
                        MM(ps[:, :],
                           _r(xt[dt][:, pi * L + lt * P: pi * L + (lt + 1) * P]),
                           _r(wq_sb[dt][:, 2 * D: 3 * D]),
                           start=(dt == 0), stop=(dt == DT - 1))
                    t = sp_pool.tile([P, D], F32R, tag=f"v{pi}{lt}",
                                     name=f"v{sp}_{pi}{lt}")
                    nc.vector.tensor_copy(t[:, :], ps[:, :])
                    v_sb[pi][lt] = t

            # S3: dots = q @ k.T  -> [i(part), m(free)], packed per i-tile
            dps = []
            for it in range(LT):
                ps = pp.tile([P, FP], F32, tag="ps", name=f"ps_d{sp}_{it}")
                for pi in range(2):
                    o = ps[:, pi * L:(pi + 1) * L]
                    for et in range(DT):
                        MM(o,
                           _r(qT[et][:, pi * L + it * P: pi * L + (it + 1) * P]),
                           _r(kT[et][:, pi * L:(pi + 1) * L]),
                           start=(et == 0), stop=(et == DT - 1))
                dps.append(ps)

            # S4: E = exp(dots * SCALE), s1 = rowsum(E) (fused accum)
            s14 = sp_pool.tile([P, 4], F32, tag="s14", name=f"s14_{sp}")
            E = []
            for it in range(LT):
                e_t = sp_pool.tile([P, FP], F32, tag=f"E{it}", name=f"E{sp}_{it}")
                for pi in range(2):
                    c = it * 2 + pi
                    sl = slice(pi * L, (pi + 1) * L)
                    nc.scalar.activation(e_t[:, sl], dps[it][:, sl], AF.Exp,
                                         scale=SCALE, accum_out=s14[:, c:c + 1])
                E.append(e_t)
            r14 = sp_pool.tile([P, 4], F32, tag="r14", name=f"r14_{sp}")
            nc.vector.reciprocal(r14[:, :], s14[:, :])

            # S5: attn_w = E*r1 + pos ; transpose -> [m(part), i(packed free)]
            aw = []
            for it in range(LT):
                t = sp_pool.tile([P, FP], F32, tag=f"aw{it}", name=f"aw{sp}_{it}")
                for pi in range(2):
                    c = it * 2 + pi
                    sl = slice(pi * L, (pi + 1) * L)
                    nc.vector.scalar_tensor_tensor(
                        t[:, sl], E[it][:, sl], r14[:, c:c + 1],
                        pos_sb[ci][it][:, sl], ALU.mult, ALU.add)
                aw.append(t)
            awT = []
            for mt in range(LT):
                ps = pp.tile([P, FP], F32, tag="ps", name=f"ps_tA{sp}_{mt}")
                for pi in range(2):
                    for it in range(LT):
                        nc.tensor.transpose(
                            ps[:, pi * L + it * P: pi * L + (it + 1) * P],
                            aw[it][:, pi * L + mt * P: pi * L + (mt + 1) * P],
                            id_sb[:, :])
                t = sp_pool.tile([P, FP], F32R, tag=f"tT{mt}", name=f"awT{sp}_{mt}")
                nc.vector.tensor_copy(t[:, :], ps[:, :])
                awT.append(t)

            # S6: h^T = relu(w1.T @ attn_w^T + b1)  -> [j(part), i(packed)]
            hT = []
            for jt in range(LT):
                ps = pp.tile([P, FP], F32, tag="ps", name=f"ps_h{sp}_{jt}")
                for mt in range(LT):
                    MM(ps[:, :],
                       _r(w1_sb[mt][:, jt * P:(jt + 1) * P]),
                       _r(awT[mt][:, :]),
                       start=(mt == 0), stop=False)
                MM(ps[:, :],
                   _r(b1_sb[0:1, jt * P:(jt + 1) * P]),
                   _r(ones_sb[0:1, :]),
                   start=False, stop=True)
                t = sp_pool.tile([P, FP], F32R, tag=f"hT{jt}", name=f"hT{sp}_{jt}")
                nc.scalar.activation(t[:, :], ps[:, :], AF.Relu)
                hT.append(t)

            # S7: w[i] = h[i,:] @ w2 ; negt = -1/(2(w+b2)^2 + 1e-6)
            wps = pp.tile([P, 8], F32, tag="ps", name=f"ps_w{sp}")
            for pi in range(2):
                for it in range(LT):
                    c = it * 2 + pi
                    for jt in range(LT):
                        MM(wps[:, 2 * c:2 * c + 2],
                           _r(hT[jt][:, pi * L + it * P: pi * L + (it + 1) * P]),
                           _r(w2_sb[jt][:, :]),
                           start=(jt == 0), stop=(jt == LT - 1))
            w4 = sp_pool.tile([P, 8], F32, tag="w4", name=f"w4_{sp}")
            nc.vector.tensor_scalar_add(w4[:, :], wps[:, :], b2_sb[:, 0:1])
            nc.vector.tensor_mul(w4[:, :], w4[:, :], w4[:, :])
            nc.vector.tensor_scalar(w4[:, :], w4[:, :], -2.0, -1e-6,
                                    ALU.mult, ALU.add)
            negt = sp_pool.tile([P, 8], F32, tag="negt", name=f"negt_{sp}")
            nc.vector.reciprocal(negt[:, :], w4[:, :])

            # S8: wg = exp(dist * negt); p2 = (E*r1)*wg; E2 = exp(p2) (+s2);
            #     attn2 = E2 * r2   (all in place in wg)
            s24 = sp_pool.tile([P, 4], F32, tag="s24", name=f"s24_{sp}")
            wg = []
            for it in range(LT):
                t = sp_pool.tile([P, FP], F32, tag=f"wg{it}", name=f"wg{sp}_{it}")
                for pi in range(2):
                    c = it * 2 + pi
                    sl = slice(pi * L, (pi + 1) * L)
                    nc.scalar.activation(t[:, sl], dist_sb[it][:, sl], AF.Exp,
                                         scale=negt[:, 2 * c:2 * c + 1])
                    nc.vector.scalar_tensor_tensor(
                        t[:, sl], E[it][:, sl], r14[:, c:c + 1], t[:, sl],
                        ALU.mult, ALU.mult)
                    nc.scalar.activation(t[:, sl], t[:, sl], AF.Exp,
                                         accum_out=s24[:, c:c + 1])
                wg.append(t)
            r24 = sp_pool.tile([P, 4], F32, tag="r24", name=f"r24_{sp}")
            nc.vector.reciprocal(r24[:, :], s24[:, :])
            for it in range(LT):
                for pi in range(2):
                    c = it * 2 + pi
                    sl = slice(pi * L, (pi + 1) * L)
                    nc.vector.tensor_scalar_mul(wg[it][:, sl], wg[it][:, sl],
                                                r24[:, c:c + 1])

            # S9: transpose attn2 -> [m(part), i(packed)]
            a2T = []
            for mt in range(LT):
                ps = pp.tile([P, FP], F32, tag="ps", name=f"ps_tB{sp}_{mt}")
                for pi in range(2):
                    for it in range(LT):
                        nc.tensor.transpose(
                            ps[:, pi * L + it * P: pi * L + (it + 1) * P],
                            wg[it][:, pi * L + mt * P: pi * L + (mt + 1) * P],
                            id_sb[:, :])
                t = sp_pool.tile([P, FP], F32R, tag=f"tT{mt}", name=f"a2T{sp}_{mt}")
                nc.vector.tensor_copy(t[:, :], ps[:, :])
                a2T.append(t)

            # S10: out^T[d, i] = sum_m v[m, d] * attn2^T[m, i]
            oT = []
            for dt in range(DT):
                ps = pp.tile([P, FP], F32, tag="ps", name=f"ps_o{sp}_{dt}")
                for pi in range(2):
                    o = ps[:, pi * L:(pi + 1) * L]
                    for mt in range(LT):
                        MM(o,
                           _r(v_sb[pi][mt][:, dt * P:(dt + 1) * P]),
                           _r(a2T[mt][:, pi * L:(pi + 1) * L]),
                           start=(mt == 0), stop=(mt == LT - 1))
                t = sp_pool.tile([P, FP], F32R, tag=f"oT{dt}", name=f"oT{sp}_{dt}")
                nc.vector.tensor_copy(t[:, :], ps[:, :])
                oT.append(t)

            # S11: y^T[d', i] = sum_d w_out[d, d'] out^T[d, i] + b_out[d']
            for ot in range(DT):
                ps = pp.tile([P, FP], F32, tag="ps", name=f"ps_y{sp}_{ot}")
                for dt in range(DT):
                    MM(ps[:, :],
                       _r(wo_sb[dt][:, ot * P:(ot + 1) * P]),
                       _r(oT[dt][:, :]),
                       start=(dt == 0), stop=(dt == DT - 1))
                yt = ypool.tile([P, FP], F32, tag=f"yT{ot}", name=f"yT{sp}_{ot}")
                nc.scalar.activation(yt[:, :], ps[:, :], AF.Identity,
                                     bias=bo_sb[ot][:, 0:1])
                nc.sync.dma_start(out=h["out"][sp, ot * P:(ot + 1) * P, :],
                                  in_=yt[:, :])


def build_nc():
    nc = bacc.Bacc("TRN2", target_bir_lowering=False, debug=False,
                   enable_asserts=False)
    h = {}
    h["x_t"] = nc.declare_dram_parameter("x_t", [NSP, D, FP], F32R, False)
    h["pos"] = nc.declare_dram_parameter("pos", [CH_PER_CORE, L, FP], F32, False)
    h["dist"] = nc.declare_dram_parameter("dist", [L, FP], F32, False)
    h["wqkv"] = nc.declare_dram_parameter("wqkv", [D, 3 * D], F32R, False)
    h["w1"] = nc.declare_dram_parameter("w1", [L, L], F32R, False)
    h["w2d"] = nc.declare_dram_parameter("w2d", [L, 2], F32R, False)
    h["b1r"] = nc.declare_dram_parameter("b1r", [1, L], F32R, False)
    h["ones"] = nc.declare_dram_parameter("ones", [1, FP], F32R, False)
    h["wout"] = nc.declare_dram_parameter("wout", [D, D], F32R, False)
    h["bout"] = nc.declare_dram_parameter("bout", [D, 1], F32, False)
    h["b2r"] = nc.declare_dram_parameter("b2r", [P, 1], F32, False)
    h["ident"] = nc.declare_dram_parameter("ident", [P, P], F32, False)
    h["out"] = nc.declare_dram_parameter("out", [NSP, D, FP], F32, True)

    with TileContext(nc) as tc:
        _emit(nc, tc, h)
    nc.compile()
    return nc


def make_in_maps(x, w_qkv, pos_emb, w1, b1, w2, b2, w_out, b_out):
    f = lambda a: np.ascontiguousarray(np.asarray(a), dtype=np.float32)
    x, w_qkv, pos_emb = f(x), f(w_qkv), f(pos_emb)
    w1, b1, w2, b2, w_out, b_out = f(w1), f(b1), f(w2), f(b2), f(w_out), f(b_out)

    idx = np.arange(L, dtype=np.float32)
    dist = (idx[None, :] - idx[:, None]) ** 2
    distp = np.ascontiguousarray(np.concatenate([dist, dist], axis=1))
    common = {
        "dist": distp,
        "wqkv": w_qkv,
        "w1": w1,
        "w2d": np.ascontiguousarray(np.concatenate([w2, w2], axis=1)),
        "b1r": np.ascontiguousarray(b1.reshape(1, L)),
        "ones": np.ones((1, FP), np.float32),
        "wout": w_out,
        "bout": np.ascontiguousarray(b_out.reshape(D, 1)),
        "b2r": np.full((P, 1), b2.reshape(-1)[0], np.float32),
        "ident": np.eye(P, dtype=np.float32),
    }
    in_maps = []
    for core in range(NCORES):
        x_t = np.empty((NSP, D, FP), np.float32)
        posm = np.empty((CH_PER_CORE, L, FP), np.float32)
        for ci in range(CH_PER_CORE):
            ch = core * CH_PER_CORE + ci
            pc = pos_emb[0, ch]
            posm[ci, :, :L] = pc
            posm[ci, :, L:] = pc
            for bp in range(B // 2):
                s = ci * (B // 2) + bp
                x_t[s, :, :L] = x[2 * bp, ch].T
                x_t[s, :, L:] = x[2 * bp + 1, ch].T
        m = dict(common)
        m["x_t"] = x_t
        m["pos"] = posm
        in_maps.append(m)
    return in_maps


def assemble_out(results):
    """results: list (per core) of dicts with 'out' [NSP, D, FP]."""
    y = np.empty((B, C, L, D), np.float32)
    for core in range(NCORES):
        o = results[core]["out"]
        for ci in range(CH_PER_CORE):
            ch = core * CH_PER_CORE + ci
            for bp in range(B // 2):
                s = ci * (B // 2) + bp
                y[2 * bp, ch] = o[s, :, :L].T
                y[2 * bp + 1, ch] = o[s, :, L:].T
    return y


_NC = None
LAST_RESULT = None


def kernel(x, w_qkv, pos_emb, w1, b1, w2, b2, w_out, b_out):
    global _NC, LAST_RESULT
    from concourse.bass_utils import run_bass_kernel_spmd

    if _NC is None:
        _NC = build_nc()
    in_maps = make_in_maps(x, w_qkv, pos_emb, w1, b1, w2, b2, w_out, b_out)
    res = run_bass_kernel_spmd(_NC, in_maps, core_ids=list(range(NCORES)))
    LAST_RESULT = res
    return assemble_out(res.results)


# revision 4
# speedup vs baseline: 1.7793x; 1.7793x over previous
"""Trainium2 Bass kernel: distance-decay double-softmax attention.

Reference computation per (b, c) pair (L=256, D=512):
    qkv  = x @ w_qkv;  q,k,v = split(qkv)
    attn = softmax(q @ k.T * D_h^-0.5)
    h    = relu((attn + pos) @ w1 + b1);  w = h @ w2 + b2
    attn2= softmax(attn * exp(-dist / (2 w^2 + 1e-6)))
    out  = (attn2 @ v) @ w_out + b_out

Host-side algebraic folds (exact):
    dots = q k^T * s = x (s Wq Wk^T) x^T         -> M = s*Wq@Wk.T
    y    = attn2 @ (v w_out) + b_out             -> Wv' = Wv@w_out
    (attn+pos) @ w1 + b1 = attn@w1 + (pos@w1+b1) -> P1[c] = pos[c]@w1+b1

Sharding: pure data parallel over the 128 (b,c) pairs -> 16 pairs/core,
packed as 8 "superpairs" (2 batch items of one channel share the free
dim, giving N=512 matmuls).  x arrives host-pretransposed; the output
leaves as y^T and is untransposed on the host.  attn / attn2 are
transposed on the PE (via identity).  All matmuls run as float32r
(full-rate fp32 storage) with fp32 PSUM accumulation.

Emission is software-pipelined across superpairs (stage A of superpair
sp is emitted before stage B of superpair sp-1) so the TensorEngine
never drains during the softmax/MLP chain and the HAM clock stays warm.
"""

import sys
import numpy as np

sys.path.insert(0, "/opt/trn_rl_repo")

import concourse.bass as bass  # noqa: E402,F401
import concourse.mybir as mybir  # noqa: E402
from concourse import bacc  # noqa: E402
from concourse.tile import TileContext  # noqa: E402

F32 = mybir.dt.float32
F32R = mybir.dt.float32r
AF = mybir.ActivationFunctionType
ALU = mybir.AluOpType

B, C, L, D = 8, 16, 256, 512
NCORES = 8
CH_PER_CORE = C // NCORES          # 2
NSP = (B // 2) * CH_PER_CORE       # 8 superpairs per core
P = 128
FP = 2 * L                         # 512: two pairs packed along free dim
DT = D // P                        # 4
LT = L // P                        # 2
SCALE = float(64 ** -0.5)          # DIM_HEAD ** -0.5


class _Ctx:
    pass


def _emit_stage_a(g, sp):
    """x load, t^T = (x M)^T, v' = x Wv', dots = t x^T, E=exp(dots)+rowsum."""
    nc, pp, sp_pool = g.nc, g.pp, g.sp_pool
    MM = nc.tensor.matmul
    st = g.state[sp] = _Ctx()

    # x^T tiles [128(d), 512(l packed)]
    xt = []
    for dt in range(DT):
        t = sp_pool.tile([P, FP], F32R, tag=f"xt{dt}", name=f"xt{sp}_{dt}")
        nc.sync.dma_start(out=t[:, :], in_=g.h["x_t"][sp, dt * P:(dt + 1) * P, :])
        xt.append(t)
    st.xt = xt

    # t^T[e, l] = sum_d M[d, e] x^T[d, l]
    tT = []
    for et in range(DT):
        ps = pp.tile([P, FP], F32, tag="ps", name=f"ps_t{sp}_{et}")
        for dt in range(DT):
            MM(ps[:, :], g.m_sb[dt][:, et * P:(et + 1) * P], xt[dt][:, :],
               start=(dt == 0), stop=(dt == DT - 1))
        t = sp_pool.tile([P, FP], F32R, tag=f"tT{et}", name=f"tT{sp}_{et}")
        nc.vector.tensor_copy(t[:, :], ps[:, :])
        tT.append(t)
    st.tT = tT

    # v'[l, e] = sum_d x^T[d, l] Wv'[d, e]   (natural layout, per pair)
    v_sb = [[None] * LT for _ in range(2)]
    for pi in range(2):
        for lt in range(LT):
            ps = pp.tile([P, D], F32, tag="ps", name=f"ps_v{sp}_{pi}{lt}")
            for dt in range(DT):
                MM(ps[:, :],
                   xt[dt][:, pi * L + lt * P: pi * L + (lt + 1) * P],
                   g.wv_sb[dt][:, :],
                   start=(dt == 0), stop=(dt == DT - 1))
            t = sp_pool.tile([P, D], F32R, tag=f"v{pi}{lt}", name=f"v{sp}_{pi}{lt}")
            nc.vector.tensor_copy(t[:, :], ps[:, :])
            v_sb[pi][lt] = t
    st.v = v_sb

    # dots[i, m] = sum_e t^T[e, i] x^T[e, m]   (scale folded into M)
    dps = []
    for it in range(LT):
        ps = pp.tile([P, FP], F32, tag="ps", name=f"ps_d{sp}_{it}")
        for pi in range(2):
            o = ps[:, pi * L:(pi + 1) * L]
            for et in range(DT):
                MM(o,
                   tT[et][:, pi * L + it * P: pi * L + (it + 1) * P],
                   xt[et][:, pi * L:(pi + 1) * L],
                   start=(et == 0), stop=(et == DT - 1))
        dps.append(ps)

    # E = exp(dots), s1 = rowsum(E)
    s14 = sp_pool.tile([P, 4], F32, tag="s14", name=f"s14_{sp}")
    E = []
    for it in range(LT):
        e_t = sp_pool.tile([P, FP], F32, tag=f"E{it}", name=f"E{sp}_{it}")
        for pi in range(2):
            c = it * 2 + pi
            sl = slice(pi * L, (pi + 1) * L)
            nc.scalar.activation(e_t[:, sl], dps[it][:, sl], AF.Exp,
                                 accum_out=s14[:, c:c + 1])
        E.append(e_t)
    st.E = E
    r14 = sp_pool.tile([P, 4], F32, tag="r14", name=f"r14_{sp}")
    nc.vector.reciprocal(r14[:, :], s14[:, :])
    st.r14 = r14


def _emit_stage_b(g, sp):
    """attn, transpose, MLP, dist-decay, softmax2, transpose, y^T, DMA out."""
    nc, pp, sp_pool = g.nc, g.pp, g.sp_pool
    MM = nc.tensor.matmul
    st = g.state[sp]
    ci = sp // (NSP // CH_PER_CORE)
    E, r14 = st.E, st.r14

    # attn = E * r1  (kept for second softmax; also the transpose source)
    attn = []
    for it in range(LT):
        t = sp_pool.tile([P, FP], F32, tag=f"at{it}", name=f"attn{sp}_{it}")
        for pi in range(2):
            c = it * 2 + pi
            sl = slice(pi * L, (pi + 1) * L)
            nc.vector.tensor_scalar_mul(t[:, sl], E[it][:, sl], r14[:, c:c + 1])
        attn.append(t)
    st.attn = attn

    # attn^T  [m(part), i(packed free)]
    aT = []
    for mt in range(LT):
        ps = pp.tile([P, FP], F32, tag="ps", name=f"ps_tA{sp}_{mt}")
        for pi in range(2):
            for it in range(LT):
                nc.tensor.transpose(
                    ps[:, pi * L + it * P: pi * L + (it + 1) * P],
                    attn[it][:, pi * L + mt * P: pi * L + (mt + 1) * P],
                    g.id_sb[:, :])
        t = sp_pool.tile([P, FP], F32R, tag=f"trT{mt}", name=f"aT{sp}_{mt}")
        nc.vector.tensor_copy(t[:, :], ps[:, :])
        aT.append(t)

    # h^T = relu(w1^T attn^T + P1^T)   [j(part), i(packed)]
    hT = []
    for jt in range(LT):
        ps = pp.tile([P, FP], F32, tag="ps", name=f"ps_h{sp}_{jt}")
        for mt in range(LT):
            MM(ps[:, :], g.w1_sb[mt][:, jt * P:(jt + 1) * P], aT[mt][:, :],
               start=(mt == 0), stop=False)
        MM(ps[:, :], g.idr_sb[:, :], g.p1_sb[ci][jt][:, :],
           start=False, stop=True)
        t = sp_pool.tile([P, FP], F32R, tag=f"hT{jt}", name=f"hT{sp}_{jt}")
        nc.scalar.activation(t[:, :], ps[:, :], AF.Relu)
        hT.append(t)

    # w[i] = h[i, :] @ w2 ; negt = -1/(2(w+b2)^2 + 1e-6)
    wps = pp.tile([P, 8], F32, tag="ps", name=f"ps_w{sp}")
    for pi in range(2):
        for it in range(LT):
            c = it * 2 + pi
            for jt in range(LT):
                MM(wps[:, 2 * c:2 * c + 2],
                   hT[jt][:, pi * L + it * P: pi * L + (it + 1) * P],
                   g.w2_sb[jt][:, :],
                   start=(jt == 0), stop=(jt == LT - 1))
    w4 = sp_pool.tile([P, 8], F32, tag="w4", name=f"w4_{sp}")
    nc.vector.tensor_scalar_add(w4[:, :], wps[:, :], g.b2_sb[:, 0:1])
    nc.vector.tensor_mul(w4[:, :], w4[:, :], w4[:, :])
    nc.vector.tensor_scalar(w4[:, :], w4[:, :], -2.0, -1e-6, ALU.mult, ALU.add)
    negt = sp_pool.tile([P, 8], F32, tag="negt", name=f"negt_{sp}")
    nc.vector.reciprocal(negt[:, :], w4[:, :])

    # wg = exp(dist * negt); p2 = attn*wg; E2 = exp(p2) (+s2); attn2 = E2*r2
    s24 = sp_pool.tile([P, 4], F32, tag="s24", name=f"s24_{sp}")
    wg = []
    for it in range(LT):
        t = sp_pool.tile([P, FP], F32, tag=f"wg{it}", name=f"wg{sp}_{it}")
        for pi in range(2):
            c = it * 2 + pi
            sl = slice(pi * L, (pi + 1) * L)
            nc.scalar.activation(t[:, sl], g.dist_sb[it][:, sl], AF.Exp,
                                 scale=negt[:, 2 * c:2 * c + 1])
        nc.vector.tensor_mul(t[:, :], st.attn[it][:, :], t[:, :])
        for pi in range(2):
            c = it * 2 + pi
            sl = slice(pi * L, (pi + 1) * L)
            nc.scalar.activation(t[:, sl], t[:, sl], AF.Exp,
                                 accum_out=s24[:, c:c + 1])
        wg.append(t)
    r24 = sp_pool.tile([P, 4], F32, tag="r24", name=f"r24_{sp}")
    nc.vector.reciprocal(r24[:, :], s24[:, :])
    for it in range(LT):
        for pi in range(2):
            c = it * 2 + pi
            sl = slice(pi * L, (pi + 1) * L)
            nc.vector.tensor_scalar_mul(wg[it][:, sl], wg[it][:, sl],
                                        r24[:, c:c + 1])

    # attn2^T [m(part), i(packed)]
    a2T = []
    for mt in range(LT):
        ps = pp.tile([P, FP], F32, tag="ps", name=f"ps_tB{sp}_{mt}")
        for pi in range(2):
            for it in range(LT):
                nc.tensor.transpose(
                    ps[:, pi * L + it * P: pi * L + (it + 1) * P],
                    wg[it][:, pi * L + mt * P: pi * L + (mt + 1) * P],
                    g.id_sb[:, :])
        t = sp_pool.tile([P, FP], F32R, tag=f"trT{mt}", name=f"a2T{sp}_{mt}")
        nc.vector.tensor_copy(t[:, :], ps[:, :])
        a2T.append(t)

    # y^T[d', i] = sum_m v'[m, d'] attn2^T[m, i] + b_out[d']
    for ot in range(DT):
        ps = pp.tile([P, FP], F32, tag="ps", name=f"ps_y{sp}_{ot}")
        for pi in range(2):
            o = ps[:, pi * L:(pi + 1) * L]
            for mt in range(LT):
                MM(o,
                   st.v[pi][mt][:, ot * P:(ot + 1) * P],
                   a2T[mt][:, pi * L:(pi + 1) * L],
                   start=(mt == 0), stop=(mt == LT - 1))
        yt = g.ypool.tile([P, FP], F32, tag=f"yT{ot}", name=f"yT{sp}_{ot}")
        nc.scalar.activation(yt[:, :], ps[:, :], AF.Identity,
                             bias=g.bo_sb[ot][:, 0:1])
        nc.sync.dma_start(out=g.h["out"][sp, ot * P:(ot + 1) * P, :],
                          in_=yt[:, :])


def _emit(nc, tc, h):
    import contextlib
    g = _Ctx()
    g.nc, g.h = nc, h
    g.state = {}

    with contextlib.ExitStack() as ex:
        cpool = ex.enter_context(tc.tile_pool(name="consts", bufs=1))
        g.sp_pool = ex.enter_context(tc.tile_pool(name="stream", bufs=2))
        g.ypool = ex.enter_context(tc.tile_pool(name="yout", bufs=1))
        g.pp = ex.enter_context(tc.tile_pool(name="ps", bufs=8, space="PSUM"))

        # ---- constants ----
        def cload(name, shape, dt_, src):
            t = cpool.tile(shape, dt_, tag=name, name=name)
            nc.sync.dma_start(out=t[:shape[0], :], in_=src)
            return t

        g.m_sb = [cload(f"m{dt}", [P, D], F32R, h["m"][dt * P:(dt + 1) * P, :])
                  for dt in range(DT)]
        g.wv_sb = [cload(f"wv{dt}", [P, D], F32R, h["wv"][dt * P:(dt + 1) * P, :])
                   for dt in range(DT)]
        g.w1_sb = [cload(f"w1_{mt}", [P, L], F32R, h["w1"][mt * P:(mt + 1) * P, :])
                   for mt in range(LT)]
        g.w2_sb = [cload(f"w2_{jt}", [P, 2], F32R, h["w2d"][jt * P:(jt + 1) * P, :])
                   for jt in range(LT)]
        g.p1_sb = [[cload(f"p1_{ci}_{jt}", [P, FP], F32R,
                          h["p1t"][ci, jt * P:(jt + 1) * P, :])
                    for jt in range(LT)] for ci in range(CH_PER_CORE)]
        g.bo_sb = [cload(f"bout{ot}", [P, 1], F32, h["bout"][ot * P:(ot + 1) * P, :])
                   for ot in range(DT)]
        g.b2_sb = cload("b2r", [P, 1], F32, h["b2r"][:, :])
        g.id_sb = cload("ident", [P, P], F32, h["ident"][:, :])
        g.idr_sb = cload("identr", [P, P], F32R, h["identr"][:, :])
        g.dist_sb = [cload(f"dist{it}", [P, FP], F32,
                           h["dist"][it * P:(it + 1) * P, :])
                     for it in range(LT)]

        # ---- software-pipelined superpair loop ----
        _emit_stage_a(g, 0)
        for sp in range(1, NSP):
            _emit_stage_a(g, sp)
            _emit_stage_b(g, sp - 1)
        _emit_stage_b(g, NSP - 1)


def build_nc():
    nc = bacc.Bacc("TRN2", target_bir_lowering=False, debug=False,
                   enable_asserts=False)
    h = {}
    h["x_t"] = nc.declare_dram_parameter("x_t", [NSP, D, FP], F32R, False)
    h["m"] = nc.declare_dram_parameter("m", [D, D], F32R, False)
    h["wv"] = nc.declare_dram_parameter("wv", [D, D], F32R, False)
    h["w1"] = nc.declare_dram_parameter("w1", [L, L], F32R, False)
    h["w2d"] = nc.declare_dram_parameter("w2d", [L, 2], F32R, False)
    h["p1t"] = nc.declare_dram_parameter("p1t", [CH_PER_CORE, L, FP], F32R, False)
    h["dist"] = nc.declare_dram_parameter("dist", [L, FP], F32, False)
    h["bout"] = nc.declare_dram_parameter("bout", [D, 1], F32, False)
    h["b2r"] = nc.declare_dram_parameter("b2r", [P, 1], F32, False)
    h["ident"] = nc.declare_dram_parameter("ident", [P, P], F32, False)
    h["identr"] = nc.declare_dram_parameter("identr", [P, P], F32R, False)
    h["out"] = nc.declare_dram_parameter("out", [NSP, D, FP], F32, True)

    with TileContext(nc) as tc:
        _emit(nc, tc, h)
    nc.compile()
    return nc


def make_in_maps(x, w_qkv, pos_emb, w1, b1, w2, b2, w_out, b_out):
    f = lambda a: np.ascontiguousarray(np.asarray(a), dtype=np.float32)
    x, w_qkv, pos_emb = f(x), f(w_qkv), f(pos_emb)
    w1, b1, w2, b2, w_out, b_out = f(w1), f(b1), f(w2), f(b2), f(w_out), f(b_out)

    wq, wk, wv = w_qkv[:, :D], w_qkv[:, D:2 * D], w_qkv[:, 2 * D:]
    m = np.ascontiguousarray((SCALE * (wq.astype(np.float64)
                                       @ wk.astype(np.float64).T))
                             .astype(np.float32))
    wvp = np.ascontiguousarray((wv.astype(np.float64)
                                @ w_out.astype(np.float64)).astype(np.float32))
    # P1[c] = pos[c] @ w1 + b1, transposed [L(j), L(i)] per channel
    p1 = pos_emb[0].astype(np.float64) @ w1.astype(np.float64) + b1
    p1t_single = np.ascontiguousarray(p1.transpose(0, 2, 1).astype(np.float32))
    idx = np.arange(L, dtype=np.float32)
    dist = (idx[None, :] - idx[:, None]) ** 2
    distp = np.ascontiguousarray(np.concatenate([dist, dist], axis=1))
    common = {
        "m": m,
        "wv": wvp,
        "w1": w1,
        "w2d": np.ascontiguousarray(np.concatenate([w2, w2], axis=1)),
        "dist": distp,
        "bout": np.ascontiguousarray(b_out.reshape(D, 1)),
        "b2r": np.full((P, 1), b2.reshape(-1)[0], np.float32),
        "ident": np.eye(P, dtype=np.float32),
        "identr": np.eye(P, dtype=np.float32),
    }
    in_maps = []
    for core in range(NCORES):
        x_t = np.empty((NSP, D, FP), np.float32)
        p1t = np.empty((CH_PER_CORE, L, FP), np.float32)
        for ci in range(CH_PER_CORE):
            ch = core * CH_PER_CORE + ci
            p1t[ci, :, :L] = p1t_single[ch]
            p1t[ci, :, L:] = p1t_single[ch]
            for bp in range(B // 2):
                s = ci * (B // 2) + bp
                x_t[s, :, :L] = x[2 * bp, ch].T
                x_t[s, :, L:] = x[2 * bp + 1, ch].T
        mcore = dict(common)
        mcore["x_t"] = x_t
        mcore["p1t"] = np.ascontiguousarray(p1t)
        in_maps.append(mcore)
    return in_maps


def assemble_out(results):
    """results: list (per core) of dicts with 'out' [NSP, D, FP]."""
    y = np.empty((B, C, L, D), np.float32)
    for core in range(NCORES):
        o = results[core]["out"]
        for ci in range(CH_PER_CORE):
            ch = core * CH_PER_CORE + ci
            for bp in range(B // 2):
                s = ci * (B // 2) + bp
                y[2 * bp, ch] = o[s, :, :L].T
                y[2 * bp + 1, ch] = o[s, :, L:].T
    return y


_NC = None
LAST_RESULT = None


def kernel(x, w_qkv, pos_emb, w1, b1, w2, b2, w_out, b_out):
    global _NC, LAST_RESULT
    from concourse.bass_utils import run_bass_kernel_spmd

    if _NC is None:
        _NC = build_nc()
    in_maps = make_in_maps(x, w_qkv, pos_emb, w1, b1, w2, b2, w_out, b_out)
    res = run_bass_kernel_spmd(_NC, in_maps, core_ids=list(range(NCORES)))
    LAST_RESULT = res
    return assemble_out(res.results)
